# revision 2
# baseline (speedup 1.0000x reference)
"""CenterLoss kernel for 8 Trainium2 NeuronCores.

Math: with d=DECAY, e=1-d, per-class mean m_c = s_c/n_c (s_c = sum of batch
features of class c, n_c = count), the reference loss decomposes exactly:

  loss*B*F = P0 + d^2*(gamma - 2*beta) - e*(2-e)*Qpair
  P0    = sum_i w'_i ||f_i||^2,  w'_i = 1 - e*(2-e)/n_i
  beta  = sum_i f_i . c_{l_i}
  gamma = sum_i ||c_{l_i}||^2
  Qpair = sum_{same-class pairs i<j} (2/n_c) f_i.f_j

Sharding: labels are sorted and split into 8 contiguous chunks of 2048
samples; each core gets its feature rows plus the compact table of the
distinct center rows its chunk references (class-dim sharding of
center_feature with label routing, per the sharding hint).

Device (per core): the compact center table is streamed to SBUF in a
column-major layout and expanded to per-sample rows with one GPSIMD
ap_gather (indices = per-sample positions in the table). beta comes from
the Tensor engine: 32 accumulated [128x128] matmuls compute
sum_p C[p,m] F[p,n] whose diagonal summed over chunks is the Frobenius
inner product <F, C>; an identity mask + DVE reduce extracts it. gamma is
one ACT Square-accumulate over the expanded centers. Per-sample norms
||f_i||^2 (needed for the w' weighting) come from a DVE square + X-reduce
over a row-major copy of the features. The host applies the w'/count
weighting to the scalar norms, sums the partial outputs, and computes the
tiny same-class pair term (~B^2/2C pairs) in float64.
"""

import os
import sys

import numpy as np

for _p in ("/opt/trn_rl_repo",):
    if _p not in sys.path and os.path.isdir(_p):
        sys.path.insert(0, _p)

import ml_dtypes

BF16 = ml_dtypes.bfloat16

B = 16384
F = 256
C = 100000
DECAY = 0.99
NCORES = 8

T = B // NCORES          # samples per core (exact split of sorted order)
NT = T // 128            # row-major feature blocks of [128, 256] per core
U = T                    # compact table rows per core (padded)
HOST_PAIR_LIMIT = 2_000_000  # beyond this, fall back to full host compute

_E = 1.0 - DECAY
_QCOEF = _E * (2.0 - _E)          # 0.0199
_D2 = DECAY * DECAY               # 0.9801

_nc_cache = None
_LAST_RESULT = None


def _ensure_ntff_hook():
    """bass_utils' trace path does `from antenv.axon_hooks import ...`
    unconditionally; some agent images lack that module. Register a stub
    (and wire the real ctypes NTFF hook when available) so trace=True /
    BASS_TRACE=1 degrades gracefully instead of crashing."""
    try:
        import antenv.axon_hooks  # noqa: F401
        return
    except ImportError:
        pass
    import types

    try:
        import antenv
    except ImportError:
        return
    mod = types.ModuleType("antenv.axon_hooks")
    holder = {"h": None}
    mod.set_axon_ntff_profile_hook = lambda h: holder.__setitem__("h", h)
    mod.get_axon_ntff_profile_hook = lambda: holder["h"]
    sys.modules["antenv.axon_hooks"] = mod
    antenv.axon_hooks = mod
    try:
        import importlib.util

        so = "/opt/axon/libaxon_pjrt.so"
        boot_py = "/root/.axon_site/trn_agent_boot/trn_boot.py"
        if os.path.exists(so) and os.path.exists(boot_py):
            spec = importlib.util.spec_from_file_location("_trn_boot_hookmod", boot_py)
            tb = importlib.util.module_from_spec(spec)
            spec.loader.exec_module(tb)
            h = tb._ntff_profile_via_ctypes(so)
            if h is not None:
                mod.set_axon_ntff_profile_hook(h)
    except Exception:
        pass


def _build_bass():
    import concourse.mybir as mybir
    import concourse.tile as tile
    from concourse import bacc

    f32 = mybir.dt.float32
    bf16 = mybir.dt.bfloat16
    i16 = mybir.dt.int16

    nc = bacc.Bacc(None)
    fidx = nc.dram_tensor("fidx", [128, T // 16], i16, kind="ExternalInput")
    ident = nc.dram_tensor("ident", [128, 128], f32, kind="ExternalInput")
    tab = nc.dram_tensor("tab", [128, U * 2], bf16, kind="ExternalInput")
    fcm = nc.dram_tensor("fcm", [128, T * 2], bf16, kind="ExternalInput")
    frm = nc.dram_tensor("frm", [128, NT * F], bf16, kind="ExternalInput")

    # one combined output: [:, :NT]=per-sample norms, [:, NT]=beta diag
    # partials, [:, NT+1]=gamma partials
    out = nc.dram_tensor("out", [128, NT + 2], f32, kind="ExternalOutput")

    NMM = (T * 2) // 128   # 32 accumulated matmuls for the beta trace

    with tile.TileContext(nc) as tc:
        with (
            tc.tile_pool(name="io", bufs=1) as io,
            tc.tile_pool(name="scr", bufs=2) as scr,
            tc.psum_pool(name="ps", bufs=1) as ps,
        ):
            idx_t = io.tile([128, T // 16], dtype=i16)
            nc.sync.dma_start(idx_t[:], fidx[:, :])
            id_t = io.tile([128, 128], dtype=f32)
            nc.sync.dma_start(id_t[:], ident[:, :])
            tab_t = io.tile([128, U * 2], dtype=bf16)
            nc.sync.dma_start(tab_t[:], tab[:, :])
            frm_t = io.tile([128, NT * F], dtype=bf16)
            for h in range(2):
                nc.sync.dma_start(
                    frm_t[:, h * NT * F // 2:(h + 1) * NT * F // 2],
                    frm[:, h * NT * F // 2:(h + 1) * NT * F // 2])
            fcm_t = io.tile([128, T * 2], dtype=bf16)
            nc.sync.dma_start(fcm_t[:], fcm[:, :])

            res = io.tile([128, NT + 2], dtype=f32)

            # Per-sample norms on DVE (row-major): two halves pipelined
            # behind the frm chunk DMAs.
            for h in range(2):
                lo, hi = h * NT * F // 2, (h + 1) * NT * F // 2
                yf = scr.tile([128, NT * F // 2], dtype=bf16, tag="yf")
                nc.vector.tensor_tensor(out=yf[:], in0=frm_t[:, lo:hi],
                                        in1=frm_t[:, lo:hi],
                                        op=mybir.AluOpType.mult)
                nc.vector.tensor_reduce(
                    out=res[:, h * NT // 2:(h + 1) * NT // 2],
                    in_=yf[:].rearrange("p (n d) -> p n d", d=F),
                    axis=mybir.AxisListType.X, op=mybir.AluOpType.add)

            # Expand compact table to per-sample centers (column-major).
            ccm_t = io.tile([128, T * 2], dtype=bf16)
            nc.gpsimd.ap_gather(
                ccm_t[:].rearrange("p (n d) -> p n d", d=2),
                tab_t[:].rearrange("p (n d) -> p n d", d=2),
                idx_t[:],
                channels=128, num_elems=U, d=2, num_idxs=T,
            )

            # beta: PE trace trick. psum[m,n] = sum_p C[p,m] F[p,n] summed
            # over 32 column chunks; the diagonal is <F, C>.
            psum_t = ps.tile([128, 128], dtype=f32)
            for k in range(NMM):
                nc.tensor.matmul(
                    out=psum_t[:],
                    lhsT=ccm_t[:, k * 128:(k + 1) * 128],
                    rhs=fcm_t[:, k * 128:(k + 1) * 128],
                    start=(k == 0), stop=(k == NMM - 1),
                )
            msk = scr.tile([128, 128], dtype=f32, tag="msk")
            nc.vector.tensor_tensor(out=msk[:], in0=psum_t[:], in1=id_t[:],
                                    op=mybir.AluOpType.mult)
            nc.vector.tensor_reduce(
                out=res[:, NT:NT + 1],
                in_=msk[:].rearrange("p (n d) -> p n d", d=128),
                axis=mybir.AxisListType.X, op=mybir.AluOpType.add)

            # gamma: one ACT Square-accumulate over the expanded centers.
            a_scr = scr.tile([128, T * 2], dtype=bf16, tag="ascr")
            nc.scalar.activation(
                a_scr[:], ccm_t[:], mybir.ActivationFunctionType.Square,
                accum_out=res[:, NT + 1:NT + 2])

            nc.sync.dma_start(out[:, :], res[:])
    nc.finalize()
    return nc


def _get_nc():
    global _nc_cache
    if _nc_cache is None:
        _nc_cache = _build_bass()
    return _nc_cache


def _wrap16(idx, n):
    """Index layout for gpsimd gathers: index j lives at [j % 16, j // 16],
    replicated to all 8 GPSIMD-core partition groups of a [128, n//16]
    int16 tile."""
    w = np.asarray(idx, dtype=np.int16).reshape(n // 16, 16).T
    return np.ascontiguousarray(np.tile(w, (8, 1)))


def _cm(x):
    """[N, 256] -> column-major [128, N*2]: out[p, 2j+e] = x[j, 2p+e]."""
    n = x.shape[0]
    t = x.T.reshape(128, 2, n).transpose(0, 2, 1)
    return np.ascontiguousarray(t.reshape(128, 2 * n))


def _host_reference(f, labels, cf):
    """Full-precision host fallback (pathological label distributions only)."""
    f64 = f.astype(np.float64)
    sums = np.zeros((C, F), np.float64)
    np.add.at(sums, labels, f64)
    counts = np.bincount(labels, minlength=C).astype(np.float64)
    mean = sums / np.maximum(counts, 1.0)[:, None]
    newc = np.where((counts > 0)[:, None],
                    DECAY * cf.astype(np.float64) + (1 - DECAY) * mean,
                    cf.astype(np.float64))
    g = newc[labels]
    return np.float32(np.mean((f64 - g) ** 2))


def kernel(batch_feature, batch_label, center_feature):
    global _LAST_RESULT
    f = np.ascontiguousarray(np.asarray(batch_feature, dtype=np.float32))
    labels = np.asarray(batch_label).astype(np.int64)
    cf = np.ascontiguousarray(np.asarray(center_feature, dtype=np.float32))

    order = np.argsort(labels, kind="stable")
    sl = labels[order]                       # sorted labels
    uniq_all, run_start, run_cnt = np.unique(sl, return_index=True,
                                             return_counts=True)
    cnt_sorted = np.repeat(run_cnt, run_cnt)  # class count per sorted sample
    wq = 1.0 - _QCOEF / cnt_sorted            # w' per sorted sample

    n_pairs_total = int(((run_cnt * (run_cnt - 1)) // 2).sum())
    if n_pairs_total > HOST_PAIR_LIMIT:
        return _host_reference(f, labels, cf)

    ident = np.eye(128, dtype=np.float32)
    in_maps = []
    for k in range(NCORES):
        seg = slice(k * T, (k + 1) * T)
        rows = order[seg]
        sl_k = sl[seg]
        uniq, pos = np.unique(sl_k, return_inverse=True)

        tab_k = np.zeros((U, F), np.float32)
        tab_k[:uniq.shape[0]] = cf[uniq]
        f_k = f[rows]

        in_maps.append({
            "fidx": _wrap16(pos.astype(np.int16), T),
            "ident": ident,
            "tab": _cm(tab_k).astype(BF16),
            "fcm": _cm(f_k).astype(BF16),
            "frm": np.ascontiguousarray(f_k.reshape(128, NT * F)).astype(BF16),
        })

    _ensure_ntff_hook()
    from concourse.bass_utils import run_bass_kernel_spmd

    nc = _get_nc()
    res = run_bass_kernel_spmd(nc, in_maps, core_ids=list(range(NCORES)))
    _LAST_RESULT = res

    p0 = beta = gamma = 0.0
    for k, r in enumerate(res.results):
        o = np.asarray(r["out"], np.float64)
        nrm = o[:, :NT].reshape(T)            # norm of sample p*NT+t at [p,t]
        wk = wq[k * T:(k + 1) * T]
        p0 += float(nrm @ wk)
        beta += float(o[:, NT].sum())
        gamma += float(o[:, NT + 1].sum())

    # same-class pair term, float64 on host (~B^2/2C pairs)
    q2 = 0.0
    dup = np.nonzero(run_cnt >= 2)[0]
    if dup.size:
        f64 = f.astype(np.float64)
        ia_l, jb_l, wt_l = [], [], []
        for r_i in dup:
            s0, n = int(run_start[r_i]), int(run_cnt[r_i])
            g = order[s0:s0 + n]
            iu, ju = np.triu_indices(n, k=1)
            ia_l.append(g[iu]); jb_l.append(g[ju])
            wt_l.append(np.full(iu.shape[0], 2.0 / n))
        ia = np.concatenate(ia_l); jb = np.concatenate(jb_l)
        wt = np.concatenate(wt_l)
        dots = np.einsum("ij,ij->i", f64[ia], f64[jb])
        q2 = float(wt @ dots)

    loss = (p0 + _D2 * (gamma - 2.0 * beta) - _QCOEF * q2) / (B * F)
    return np.float32(loss)
